# revision 13
# baseline (speedup 1.0000x reference)
"""Trainium2 Bass kernel for a 2-layer GCN (GRACE encoder) on 8 NeuronCores.

Math (per layer, from the reference):
    h   = Z @ W
    deg = bincount(dst)            (self-loops included in edge list)
    dinv = deg^-1/2
    out = PReLU(segment_sum(h[src] * dinv[src] * dinv[dst], dst) + b)

We use dinv[s]*h[s] = ((dinv*Z) @ W)[s] =: P[s], so the per-edge work is a
pure row-gather of P plus a segment-sum, and all scaling is per-node:
    out = PReLU(dinv * segment_sum(P[src], dst) + b)

Sharding: dst-partitioned. Core c owns dst rows [c*12544, (c+1)*12544).
Each core computes P for its own rows, an AllGather makes the full P table
visible everywhere, and the scatter (segment-sum) is done with one-hot
selection matmuls accumulating in PSUM, 128 edges per matmul.

Tables travel as f16 (P2 zero-padded to 128 features so every gathered row
is 256 B). Edge messages are fetched with batched `dma_gather` custom DMA
instructions — int16 indices limit each gather to a 25088-row quarter of
the table, so each dst block does 4 gathers (edges are quarter-sorted).
PSUM accumulation and the PReLU epilogue stay f32.
"""

import sys

for p in ("/opt/trn_rl_repo", "/opt/trn_rl_repo/concourse"):
    if p not in sys.path:
        sys.path.insert(0, p)

import numpy as np

import concourse.bass as bass
import concourse.bacc as bacc
import concourse.tile as tile
from concourse import mybir
from concourse.bass import ds
from concourse.bass_utils import run_bass_kernel_spmd
from concourse.masks import make_identity

N = 100000
E = 1600000
FIN = 128
HID = 128
FOUT = 64
NCORES = 8
BPC = 12544          # dst rows per core (padded); 8 * 12544 = 100352
NPAD = NCORES * BPC
NBLK = BPC // 128    # 98 dst blocks of 128 per core
PCH = 128            # edges per matmul chunk
NQ = 4               # source quarters (int16 gather index limit)
QS = 32768           # rows per quarter (pow2 -> shift/mask indexing)
QROWS = (32768, 32768, 32768, NPAD - 3 * 32768)   # last quarter is short

F16 = mybir.dt.float16
F32 = mybir.dt.float32
U8 = mybir.dt.uint8
I16 = mybir.dt.int16

_cache = {}


def _preprocess(edge_index):
    """Sort edges by (dst block, src); group per (dst block, src quarter)
    with per-quarter chunk counts Kq (SPMD + loop-friendly). Vectorized."""
    loops = np.arange(N, dtype=np.int32)
    src = np.concatenate([edge_index[0], loops])
    dst = np.concatenate([edge_index[1], loops])
    deg = np.bincount(dst, minlength=N).astype(np.float32)
    dinv = np.zeros(NPAD, np.float32)
    dinv[:N] = np.where(deg > 0, 1.0 / np.sqrt(deg), 0.0)

    blk = dst >> 7                              # global 128-row dst block id
    key = (blk << 17) | src                     # int32; sorts by (blk, src)
    order = np.argsort(key, kind="stable")
    ks = key[order]
    src_s = ks & 0x1FFFF
    blk_s = ks >> 17
    ld_s = (dst & 127).astype(np.uint8)[order]
    qr_s = src_s >> 15                          # src quarter (QS = 2^15)
    loc_s = (src_s & (QS - 1)).astype(np.int16)

    nblk_glob = NPAD // 128                     # 784
    g4 = (blk_s << 2) | qr_s                    # sorted ascending
    counts4 = np.bincount(g4, minlength=nblk_glob * NQ)
    cmax = counts4.reshape(nblk_glob, NQ).max(axis=0)
    Kq = tuple(max(1, int(np.ceil(c / PCH))) for c in cmax)
    offq = np.zeros(NQ, np.int32)
    offq[1:] = np.cumsum(Kq)[:-1]
    CB = int(sum(Kq))                           # chunks per dst block
    C = NBLK * CB                               # chunks per core per layer

    bstart = np.zeros(nblk_glob * NQ + 1, np.int32)
    bstart[1:] = np.cumsum(counts4)
    ne = len(src_s)
    rank = np.arange(ne, dtype=np.int32) - bstart[g4]
    core = blk_s // NBLK
    j = blk_s - core * NBLK
    pos = core * (C * PCH) + (j * CB + offq[qr_s]) * PCH + rank

    sa = np.zeros(NCORES * C * PCH, np.int16)
    la = np.full(NCORES * C * PCH, 255, np.uint8)
    sa[pos] = loc_s
    la[pos] = ld_s

    # idxs: per gather call (j,qr) of n=Kq[qr]*128 edges, elem i -> [i%16, i//16]
    idxs_dev = np.empty((NCORES, 16, C * 8), np.int16)
    A = sa.reshape(NCORES, NBLK, CB * PCH)
    out4 = idxs_dev.reshape(NCORES, 16, NBLK, CB * 8)
    for qr in range(NQ):
        o, k = int(offq[qr]), Kq[qr]
        seg = (A[:, :, o * PCH:(o + k) * PCH]
               .reshape(NCORES, NBLK, k * 8, 16).transpose(0, 3, 1, 2))
        out4[:, :, :, o * 8:(o + k) * 8] = seg
    # ldst: [core, C, 128] -> [core, 128, C] (partition = slot in chunk)
    ldst_dev = np.ascontiguousarray(
        la.reshape(NCORES, C, PCH).swapaxes(1, 2))

    return dinv, idxs_dev, ldst_dev, Kq, C


def _build(Kq, a_val):
    """Build the SPMD Bass program (identical on all cores)."""
    CB = int(sum(Kq))
    C = NBLK * CB
    offq = [0] * NQ
    for qr in range(1, NQ):
        offq[qr] = offq[qr - 1] + Kq[qr - 1]
    nc = bacc.Bacc("TRN2", target_bir_lowering=False, debug=False,
                   num_devices=NCORES)

    xT = nc.dram_tensor("xT", [128, BPC], F16, kind="ExternalInput")
    idxs = nc.dram_tensor("idxs", [16, C * 8], I16, kind="ExternalInput")
    ldst = nc.dram_tensor("ldst", [128, C], U8, kind="ExternalInput")
    W1 = nc.dram_tensor("W1", [FIN, HID], F16, kind="ExternalInput")
    W2 = nc.dram_tensor("W2", [HID, 128], F16, kind="ExternalInput")  # 0-padded
    b1 = nc.dram_tensor("b1", [128, HID], F32, kind="ExternalInput")
    b2 = nc.dram_tensor("b2", [128, FOUT], F32, kind="ExternalInput")
    dinvb = nc.dram_tensor("dinvb", [128, NBLK], F32, kind="ExternalInput")
    iota = nc.dram_tensor("iota", [128, 128], U8, kind="ExternalInput")
    out = nc.dram_tensor("out", [BPC, FOUT], F16, kind="ExternalOutput")

    P1_my = nc.dram_tensor("P1_my", [BPC, HID], F16, kind="Internal")
    P1_full = nc.dram_tensor("P1_full", [NPAD, HID], F16, kind="Internal",
                             addr_space="Shared")
    P2_my = nc.dram_tensor("P2_my", [BPC, 128], F16, kind="Internal")
    P2_full = nc.dram_tensor("P2_full", [NPAD, 128], F16, kind="Internal",
                             addr_space="Shared")

    with tile.TileContext(nc) as tc:
        with (
            tc.tile_pool(name="persist", bufs=1) as pp,
            tc.tile_pool(name="work", bufs=4) as wp,
            tc.tile_pool(name="gath", bufs=4) as gp,
            tc.tile_pool(name="psA", bufs=2, space="PSUM") as psA,
            tc.tile_pool(name="psB", bufs=2, space="PSUM") as psB,
        ):
            # ---- persistent SBUF state ----
            xT_sb = pp.tile([128, BPC], F16)
            nc.sync.dma_start(out=xT_sb[:], in_=xT[:])
            idx_sb = pp.tile([128, C * 8], I16)
            nc.sync.dma_start(out=idx_sb[0:16, :], in_=idxs[:])
            # replicate the 16-partition wrap across all 8 gpsimd core groups
            nc.sync.dma_start(out=idx_sb[16:32, :], in_=idx_sb[0:16, :])
            nc.sync.dma_start(out=idx_sb[32:64, :], in_=idx_sb[0:32, :])
            nc.sync.dma_start(out=idx_sb[64:128, :], in_=idx_sb[0:64, :])
            ldst_sb = pp.tile([128, C], U8)
            nc.sync.dma_start(out=ldst_sb[:], in_=ldst[:])
            W1_sb = pp.tile([FIN, HID], F16)
            nc.sync.dma_start(out=W1_sb[:], in_=W1[:])
            W2_sb = pp.tile([HID, 128], F16)
            nc.sync.dma_start(out=W2_sb[:], in_=W2[:])
            b1_sb = pp.tile([128, HID], F32)
            nc.sync.dma_start(out=b1_sb[:], in_=b1[:])
            b2_sb = pp.tile([128, FOUT], F32)
            nc.sync.dma_start(out=b2_sb[:], in_=b2[:])
            dinv_sb = pp.tile([128, NBLK], F32)
            nc.sync.dma_start(out=dinv_sb[:], in_=dinvb[:])
            iota_sb = pp.tile([128, 128], U8)
            nc.sync.dma_start(out=iota_sb[:], in_=iota[:])
            ident_sb = pp.tile([128, 128], F16)
            make_identity(nc, ident_sb[:])
            h1T_sb = pp.tile([128, BPC], F16)   # transposed layer-1 output

            # ---- phase A: P1 = dinv * (x @ W1), own shard ----
            # (python-unrolled: matmul lhsT cannot take a register offset)
            for j in range(NBLK):
                ps = psA.tile([128, HID], F32, tag="pcomp")
                nc.tensor.matmul(out=ps[:], lhsT=xT_sb[:, j * 128:(j + 1) * 128],
                                 rhs=W1_sb[:], start=True, stop=True)
                p1t = wp.tile([128, HID], F16, tag="ptile")
                nc.vector.tensor_scalar_mul(p1t[:], ps[:], dinv_sb[:, j:j + 1])
                nc.sync.dma_start(out=P1_my[j * 128:(j + 1) * 128, :], in_=p1t[:])

            nc.gpsimd.collective_compute(
                "AllGather", mybir.AluOpType.bypass,
                replica_groups=[list(range(NCORES))],
                ins=[P1_my[:]], outs=[P1_full[:]],
            )

            def gather_block(i, table, msgtag):
                msg = gp.tile([128, CB * 128], F16, tag=msgtag)
                for qr in range(NQ):
                    k = Kq[qr]
                    o = offq[qr]
                    nc.gpsimd.dma_gather(
                        out_ap=msg[:, o * 128:(o + k) * 128]
                            .rearrange("p (a b) -> p a b", a=k),
                        in_ap=table[qr * QS:qr * QS + QROWS[qr], :],
                        idxs_ap=idx_sb[:, ds(i * (CB * 8) + o * 8, k * 8)],
                        num_idxs=k * 128,
                        num_idxs_reg=k * 128,
                        elem_size=128,
                    )
                return msg

            def sel_block(i):
                selg = wp.tile([128, CB * 128], F16, tag="selg")
                nc.vector.tensor_tensor(
                    out=selg[:].rearrange("p (a b) -> p a b", a=CB),
                    in0=ldst_sb[:, ds(i * CB, CB), None]
                        .to_broadcast([128, CB, 128]),
                    in1=iota_sb[:, None, :].to_broadcast([128, CB, 128]),
                    op=mybir.AluOpType.is_equal)
                return selg

            # ---- phase B: layer-1 gather + scatter matmuls ----
            with tc.For_i(0, NBLK) as i:
                selg = sel_block(i)
                msg = gather_block(i, P1_full, "msg1")
                agg = psA.tile([128, HID], F32, tag="agg")
                for q in range(CB):
                    nc.tensor.matmul(out=agg[:], lhsT=selg[:, q * 128:(q + 1) * 128],
                                     rhs=msg[:, q * 128:q * 128 + HID],
                                     start=(q == 0), stop=(q == CB - 1))
                # finalize: h1 = PReLU(dinv*agg + b1)
                z = wp.tile([128, HID], F32, tag="z1")
                nc.vector.tensor_scalar_mul(z[:], agg[:], dinv_sb[:, ds(i, 1)])
                nc.vector.tensor_tensor(out=z[:], in0=z[:], in1=b1_sb[:],
                                        op=mybir.AluOpType.add)
                za = wp.tile([128, HID], F32, tag="za1")
                nc.vector.tensor_scalar_mul(za[:], z[:], float(a_val))
                h1 = wp.tile([128, HID], F16, tag="h1")
                nc.vector.tensor_tensor(out=h1[:], in0=z[:], in1=za[:],
                                        op=mybir.AluOpType.max)
                # transpose for the layer-2 P matmul
                pt = psB.tile([128, 128], F16, tag="tpose")
                nc.tensor.transpose(out=pt[:], in_=h1[:], identity=ident_sb[:])
                nc.vector.tensor_copy(h1T_sb[:, ds(i * 128, 128)], pt[:])

            # ---- phase C: P2 = dinv * (h1 @ W2pad), own shard ----
            # (python-unrolled: matmul lhsT cannot take a register offset)
            for j in range(NBLK):
                ps = psA.tile([128, 128], F32, tag="pcomp")
                nc.tensor.matmul(out=ps[:], lhsT=h1T_sb[:, j * 128:(j + 1) * 128],
                                 rhs=W2_sb[:], start=True, stop=True)
                p2t = wp.tile([128, 128], F16, tag="ptile")
                nc.vector.tensor_scalar_mul(p2t[:], ps[:], dinv_sb[:, j:j + 1])
                nc.sync.dma_start(out=P2_my[j * 128:(j + 1) * 128, :], in_=p2t[:])

            nc.gpsimd.collective_compute(
                "AllGather", mybir.AluOpType.bypass,
                replica_groups=[list(range(NCORES))],
                ins=[P2_my[:]], outs=[P2_full[:]],
            )

            # ---- phase D: layer-2 gather + scatter + finalize ----
            with tc.For_i(0, NBLK) as i:
                selg = sel_block(i)
                msg = gather_block(i, P2_full, "msg2")
                agg = psA.tile([128, FOUT], F32, tag="agg")
                for q in range(CB):
                    nc.tensor.matmul(out=agg[:], lhsT=selg[:, q * 128:(q + 1) * 128],
                                     rhs=msg[:, q * 128:q * 128 + FOUT],
                                     start=(q == 0), stop=(q == CB - 1))
                z = wp.tile([128, FOUT], F32, tag="z2")
                nc.vector.tensor_scalar_mul(z[:], agg[:], dinv_sb[:, ds(i, 1)])
                nc.vector.tensor_tensor(out=z[:], in0=z[:], in1=b2_sb[:],
                                        op=mybir.AluOpType.add)
                za = wp.tile([128, FOUT], F32, tag="za2")
                nc.vector.tensor_scalar_mul(za[:], z[:], float(a_val))
                yo = wp.tile([128, FOUT], F16, tag="yo")
                nc.vector.tensor_tensor(out=yo[:], in0=z[:], in1=za[:],
                                        op=mybir.AluOpType.max)
                nc.sync.dma_start(out=out[ds(i * 128, 128), :], in_=yo[:])

    nc.compile()
    return nc


def _stage_x(x, W1, b1, W2, b2):
    """Edge-independent staging (overlaps with _preprocess in a thread)."""
    x16 = x.astype(np.float16)
    x_pad = np.zeros((NPAD, FIN), np.float16)
    x_pad[:N] = x16
    xT_list = [np.ascontiguousarray(x_pad[c * BPC:(c + 1) * BPC].T)
               for c in range(NCORES)]
    W1d = W1.astype(np.float16)
    W2d = np.zeros((HID, 128), np.float16)
    W2d[:, :FOUT] = W2.astype(np.float16)
    b1d = np.broadcast_to(b1, (128, HID)).astype(np.float32).copy()
    b2d = np.broadcast_to(b2, (128, FOUT)).astype(np.float32).copy()
    iota_np = np.tile(np.arange(128, dtype=np.uint8), (128, 1)).copy()
    return xT_list, W1d, W2d, b1d, b2d, iota_np


def kernel(x, edge_index, W1, b1, W2, b2, a, _want_results=False, _trace=False):
    import threading

    x = np.asarray(x, np.float32)
    edge_index = np.asarray(edge_index, np.int32)
    pre = {}

    def _run_pre():
        pre["r"] = _preprocess(edge_index)

    th = threading.Thread(target=_run_pre)
    th.start()
    xT_list, W1d, W2d, b1d, b2d, iota_np = _stage_x(
        x, np.asarray(W1, np.float32), np.asarray(b1, np.float32),
        np.asarray(W2, np.float32), np.asarray(b2, np.float32))
    th.join()
    dinv, idxs_dev, ldst_dev, Kq, C = pre["r"]

    key = (Kq, float(a))
    if key not in _cache:
        _cache[key] = _build(Kq, float(a))
    nc = _cache[key]
    in_maps = []
    for c in range(NCORES):
        lo, hi = c * BPC, (c + 1) * BPC
        in_maps.append({
            "xT": xT_list[c],
            "idxs": idxs_dev[c],
            "ldst": ldst_dev[c],
            "W1": W1d, "W2": W2d, "b1": b1d, "b2": b2d,
            "dinvb": np.ascontiguousarray(dinv[lo:hi].reshape(NBLK, 128).T),
            "iota": iota_np,
        })
    res = run_bass_kernel_spmd(nc, in_maps, core_ids=list(range(NCORES)),
                               trace=_trace)
    outs = [res.results[c]["out"] for c in range(NCORES)]
    full = np.concatenate(outs, axis=0)[:N].astype(np.float32)
    if _want_results:
        return full, res
    return full


# revision 14
# speedup vs baseline: 1.7284x; 1.7284x over previous
"""Trainium2 Bass kernel for a 2-layer GCN (GRACE encoder) on 8 NeuronCores.

Math (per layer, from the reference):
    h   = Z @ W
    deg = bincount(dst)            (self-loops included in edge list)
    dinv = deg^-1/2
    out = PReLU(segment_sum(h[src] * dinv[src] * dinv[dst], dst) + b)

We use dinv[s]*h[s] = ((dinv*Z) @ W)[s] =: P[s], so the per-edge work is a
pure row-gather of P plus a segment-sum, and all scaling is per-node:
    out = PReLU(dinv * segment_sum(P[src], dst) + b)

Sharding: dst-partitioned. Core c owns dst rows [c*12544, (c+1)*12544).
Each core computes P for its own rows, an AllGather makes the full P table
visible everywhere, and the scatter (segment-sum) is done with one-hot
selection matmuls accumulating in PSUM, 128 edges per matmul.

Tables travel as f16 (P2 zero-padded to 128 features so every gathered row
is 256 B). Edge messages are fetched with batched `dma_gather` custom DMA
instructions — int16 indices limit each gather to a 25088-row quarter of
the table, so each dst block does 4 gathers (edges are quarter-sorted).
PSUM accumulation and the PReLU epilogue stay f32.
"""

import sys

for p in ("/opt/trn_rl_repo", "/opt/trn_rl_repo/concourse"):
    if p not in sys.path:
        sys.path.insert(0, p)

import numpy as np

import jax

# Persistent XLA compilation cache: repeated kernel() calls re-trace a fresh
# closure inside run_bass_kernel_spmd; the disk cache turns the per-call
# backend compile (XLA + walrus NEFF wrap) into a hash lookup.
jax.config.update("jax_compilation_cache_dir", "/tmp/jax_comp_cache_gcn")
jax.config.update("jax_persistent_cache_min_entry_size_bytes", 0)
jax.config.update("jax_persistent_cache_min_compile_time_secs", 0)

import concourse.bass as bass
import concourse.bacc as bacc
import concourse.tile as tile
from concourse import mybir
from concourse.bass import ds
from concourse.bass_utils import run_bass_kernel_spmd
from concourse.masks import make_identity

N = 100000
E = 1600000
FIN = 128
HID = 128
FOUT = 64
NCORES = 8
BPC = 12544          # dst rows per core (padded); 8 * 12544 = 100352
NPAD = NCORES * BPC
NBLK = BPC // 128    # 98 dst blocks of 128 per core
PCH = 128            # edges per matmul chunk
NQ = 4               # source quarters (int16 gather index limit)
QS = 32768           # rows per quarter (pow2 -> shift/mask indexing)
QROWS = (32768, 32768, 32768, NPAD - 3 * 32768)   # last quarter is short

F16 = mybir.dt.float16
F32 = mybir.dt.float32
U8 = mybir.dt.uint8
I16 = mybir.dt.int16

_cache = {}


def _preprocess(edge_index):
    """Sort edges by (dst block, src); group per (dst block, src quarter)
    with per-quarter chunk counts Kq (SPMD + loop-friendly). Vectorized."""
    loops = np.arange(N, dtype=np.int32)
    src = np.concatenate([edge_index[0], loops])
    dst = np.concatenate([edge_index[1], loops])
    deg = np.bincount(dst, minlength=N).astype(np.float32)
    dinv = np.zeros(NPAD, np.float32)
    dinv[:N] = np.where(deg > 0, 1.0 / np.sqrt(deg), 0.0)

    blk = dst >> 7                              # global 128-row dst block id
    key = (blk << 17) | src                     # int32; sorts by (blk, src)
    order = np.argsort(key, kind="stable")
    ks = key[order]
    src_s = ks & 0x1FFFF
    blk_s = ks >> 17
    ld_s = (dst & 127).astype(np.uint8)[order]
    qr_s = src_s >> 15                          # src quarter (QS = 2^15)
    loc_s = (src_s & (QS - 1)).astype(np.int16)

    nblk_glob = NPAD // 128                     # 784
    g4 = (blk_s << 2) | qr_s                    # sorted ascending
    counts4 = np.bincount(g4, minlength=nblk_glob * NQ)
    cmax = counts4.reshape(nblk_glob, NQ).max(axis=0)
    Kq = tuple(max(1, int(np.ceil(c / PCH))) for c in cmax)
    offq = np.zeros(NQ, np.int32)
    offq[1:] = np.cumsum(Kq)[:-1]
    CB = int(sum(Kq))                           # chunks per dst block
    C = NBLK * CB                               # chunks per core per layer

    bstart = np.zeros(nblk_glob * NQ + 1, np.int32)
    bstart[1:] = np.cumsum(counts4)
    ne = len(src_s)
    rank = np.arange(ne, dtype=np.int32) - bstart[g4]
    core = blk_s // NBLK
    j = blk_s - core * NBLK
    pos = core * (C * PCH) + (j * CB + offq[qr_s]) * PCH + rank

    sa = np.zeros(NCORES * C * PCH, np.int16)
    la = np.full(NCORES * C * PCH, 255, np.uint8)
    sa[pos] = loc_s
    la[pos] = ld_s

    # idxs: per gather call (j,qr) of n=Kq[qr]*128 edges, elem i -> [i%16, i//16]
    idxs_dev = np.empty((NCORES, 16, C * 8), np.int16)
    A = sa.reshape(NCORES, NBLK, CB * PCH)
    out4 = idxs_dev.reshape(NCORES, 16, NBLK, CB * 8)
    for qr in range(NQ):
        o, k = int(offq[qr]), Kq[qr]
        seg = (A[:, :, o * PCH:(o + k) * PCH]
               .reshape(NCORES, NBLK, k * 8, 16).transpose(0, 3, 1, 2))
        out4[:, :, :, o * 8:(o + k) * 8] = seg
    # ldst: [core, C, 128] -> [core, 128, C] (partition = slot in chunk)
    ldst_dev = np.ascontiguousarray(
        la.reshape(NCORES, C, PCH).swapaxes(1, 2))

    return dinv, idxs_dev, ldst_dev, Kq, C


def _build(Kq, a_val):
    """Build the SPMD Bass program (identical on all cores)."""
    CB = int(sum(Kq))
    C = NBLK * CB
    offq = [0] * NQ
    for qr in range(1, NQ):
        offq[qr] = offq[qr - 1] + Kq[qr - 1]
    nc = bacc.Bacc("TRN2", target_bir_lowering=False, debug=False,
                   num_devices=NCORES)

    xT = nc.dram_tensor("xT", [128, BPC], F16, kind="ExternalInput")
    idxs = nc.dram_tensor("idxs", [16, C * 8], I16, kind="ExternalInput")
    ldst = nc.dram_tensor("ldst", [128, C], U8, kind="ExternalInput")
    W1 = nc.dram_tensor("W1", [FIN, HID], F16, kind="ExternalInput")
    W2 = nc.dram_tensor("W2", [HID, 128], F16, kind="ExternalInput")  # 0-padded
    b1 = nc.dram_tensor("b1", [128, HID], F32, kind="ExternalInput")
    b2 = nc.dram_tensor("b2", [128, FOUT], F32, kind="ExternalInput")
    dinvb = nc.dram_tensor("dinvb", [128, NBLK], F32, kind="ExternalInput")
    iota = nc.dram_tensor("iota", [128, 128], U8, kind="ExternalInput")
    out = nc.dram_tensor("out", [BPC, FOUT], F16, kind="ExternalOutput")

    P1_my = nc.dram_tensor("P1_my", [BPC, HID], F16, kind="Internal")
    P1_full = nc.dram_tensor("P1_full", [NPAD, HID], F16, kind="Internal",
                             addr_space="Shared")
    P2_my = nc.dram_tensor("P2_my", [BPC, 128], F16, kind="Internal")
    P2_full = nc.dram_tensor("P2_full", [NPAD, 128], F16, kind="Internal",
                             addr_space="Shared")

    with tile.TileContext(nc) as tc:
        with (
            tc.tile_pool(name="persist", bufs=1) as pp,
            tc.tile_pool(name="work", bufs=4) as wp,
            tc.tile_pool(name="gath", bufs=4) as gp,
            tc.tile_pool(name="psA", bufs=2, space="PSUM") as psA,
            tc.tile_pool(name="psB", bufs=2, space="PSUM") as psB,
        ):
            # ---- persistent SBUF state ----
            xT_sb = pp.tile([128, BPC], F16)
            nc.sync.dma_start(out=xT_sb[:], in_=xT[:])
            idx_sb = pp.tile([128, C * 8], I16)
            nc.sync.dma_start(out=idx_sb[0:16, :], in_=idxs[:])
            # replicate the 16-partition wrap across all 8 gpsimd core groups
            nc.sync.dma_start(out=idx_sb[16:32, :], in_=idx_sb[0:16, :])
            nc.sync.dma_start(out=idx_sb[32:64, :], in_=idx_sb[0:32, :])
            nc.sync.dma_start(out=idx_sb[64:128, :], in_=idx_sb[0:64, :])
            ldst_sb = pp.tile([128, C], U8)
            nc.sync.dma_start(out=ldst_sb[:], in_=ldst[:])
            W1_sb = pp.tile([FIN, HID], F16)
            nc.sync.dma_start(out=W1_sb[:], in_=W1[:])
            W2_sb = pp.tile([HID, 128], F16)
            nc.sync.dma_start(out=W2_sb[:], in_=W2[:])
            b1_sb = pp.tile([128, HID], F32)
            nc.sync.dma_start(out=b1_sb[:], in_=b1[:])
            b2_sb = pp.tile([128, FOUT], F32)
            nc.sync.dma_start(out=b2_sb[:], in_=b2[:])
            dinv_sb = pp.tile([128, NBLK], F32)
            nc.sync.dma_start(out=dinv_sb[:], in_=dinvb[:])
            iota_sb = pp.tile([128, 128], U8)
            nc.sync.dma_start(out=iota_sb[:], in_=iota[:])
            ident_sb = pp.tile([128, 128], F16)
            make_identity(nc, ident_sb[:])
            h1T_sb = pp.tile([128, BPC], F16)   # transposed layer-1 output

            # ---- phase A: P1 = dinv * (x @ W1), own shard ----
            # (python-unrolled: matmul lhsT cannot take a register offset)
            for j in range(NBLK):
                ps = psA.tile([128, HID], F32, tag="pcomp")
                nc.tensor.matmul(out=ps[:], lhsT=xT_sb[:, j * 128:(j + 1) * 128],
                                 rhs=W1_sb[:], start=True, stop=True)
                p1t = wp.tile([128, HID], F16, tag="ptile")
                nc.vector.tensor_scalar_mul(p1t[:], ps[:], dinv_sb[:, j:j + 1])
                nc.sync.dma_start(out=P1_my[j * 128:(j + 1) * 128, :], in_=p1t[:])

            nc.gpsimd.collective_compute(
                "AllGather", mybir.AluOpType.bypass,
                replica_groups=[list(range(NCORES))],
                ins=[P1_my[:]], outs=[P1_full[:]],
            )

            def gather_block(i, table, msgtag):
                msg = gp.tile([128, CB * 128], F16, tag=msgtag)
                for qr in range(NQ):
                    k = Kq[qr]
                    o = offq[qr]
                    nc.gpsimd.dma_gather(
                        out_ap=msg[:, o * 128:(o + k) * 128]
                            .rearrange("p (a b) -> p a b", a=k),
                        in_ap=table[qr * QS:qr * QS + QROWS[qr], :],
                        idxs_ap=idx_sb[:, ds(i * (CB * 8) + o * 8, k * 8)],
                        num_idxs=k * 128,
                        num_idxs_reg=k * 128,
                        elem_size=128,
                    )
                return msg

            def sel_block(i):
                selg = wp.tile([128, CB * 128], F16, tag="selg")
                nc.vector.tensor_tensor(
                    out=selg[:].rearrange("p (a b) -> p a b", a=CB),
                    in0=ldst_sb[:, ds(i * CB, CB), None]
                        .to_broadcast([128, CB, 128]),
                    in1=iota_sb[:, None, :].to_broadcast([128, CB, 128]),
                    op=mybir.AluOpType.is_equal)
                return selg

            # ---- phase B: layer-1 gather + scatter matmuls ----
            with tc.For_i(0, NBLK) as i:
                selg = sel_block(i)
                msg = gather_block(i, P1_full, "msg1")
                agg = psA.tile([128, HID], F32, tag="agg")
                for q in range(CB):
                    nc.tensor.matmul(out=agg[:], lhsT=selg[:, q * 128:(q + 1) * 128],
                                     rhs=msg[:, q * 128:q * 128 + HID],
                                     start=(q == 0), stop=(q == CB - 1))
                # finalize: h1 = PReLU(dinv*agg + b1)
                z = wp.tile([128, HID], F32, tag="z1")
                nc.vector.tensor_scalar_mul(z[:], agg[:], dinv_sb[:, ds(i, 1)])
                nc.vector.tensor_tensor(out=z[:], in0=z[:], in1=b1_sb[:],
                                        op=mybir.AluOpType.add)
                za = wp.tile([128, HID], F32, tag="za1")
                nc.vector.tensor_scalar_mul(za[:], z[:], float(a_val))
                h1 = wp.tile([128, HID], F16, tag="h1")
                nc.vector.tensor_tensor(out=h1[:], in0=z[:], in1=za[:],
                                        op=mybir.AluOpType.max)
                # transpose for the layer-2 P matmul
                pt = psB.tile([128, 128], F16, tag="tpose")
                nc.tensor.transpose(out=pt[:], in_=h1[:], identity=ident_sb[:])
                nc.vector.tensor_copy(h1T_sb[:, ds(i * 128, 128)], pt[:])

            # ---- phase C: P2 = dinv * (h1 @ W2pad), own shard ----
            # (python-unrolled: matmul lhsT cannot take a register offset)
            for j in range(NBLK):
                ps = psA.tile([128, 128], F32, tag="pcomp")
                nc.tensor.matmul(out=ps[:], lhsT=h1T_sb[:, j * 128:(j + 1) * 128],
                                 rhs=W2_sb[:], start=True, stop=True)
                p2t = wp.tile([128, 128], F16, tag="ptile")
                nc.vector.tensor_scalar_mul(p2t[:], ps[:], dinv_sb[:, j:j + 1])
                nc.sync.dma_start(out=P2_my[j * 128:(j + 1) * 128, :], in_=p2t[:])

            nc.gpsimd.collective_compute(
                "AllGather", mybir.AluOpType.bypass,
                replica_groups=[list(range(NCORES))],
                ins=[P2_my[:]], outs=[P2_full[:]],
            )

            # ---- phase D: layer-2 gather + scatter + finalize ----
            with tc.For_i(0, NBLK) as i:
                selg = sel_block(i)
                msg = gather_block(i, P2_full, "msg2")
                agg = psA.tile([128, FOUT], F32, tag="agg")
                for q in range(CB):
                    nc.tensor.matmul(out=agg[:], lhsT=selg[:, q * 128:(q + 1) * 128],
                                     rhs=msg[:, q * 128:q * 128 + FOUT],
                                     start=(q == 0), stop=(q == CB - 1))
                z = wp.tile([128, FOUT], F32, tag="z2")
                nc.vector.tensor_scalar_mul(z[:], agg[:], dinv_sb[:, ds(i, 1)])
                nc.vector.tensor_tensor(out=z[:], in0=z[:], in1=b2_sb[:],
                                        op=mybir.AluOpType.add)
                za = wp.tile([128, FOUT], F32, tag="za2")
                nc.vector.tensor_scalar_mul(za[:], z[:], float(a_val))
                yo = wp.tile([128, FOUT], F16, tag="yo")
                nc.vector.tensor_tensor(out=yo[:], in0=z[:], in1=za[:],
                                        op=mybir.AluOpType.max)
                nc.sync.dma_start(out=out[ds(i * 128, 128), :], in_=yo[:])

    nc.compile()
    return nc


def _stage_x(x, W1, b1, W2, b2):
    """Edge-independent staging (overlaps with _preprocess in a thread)."""
    x16 = x.astype(np.float16)
    x_pad = np.zeros((NPAD, FIN), np.float16)
    x_pad[:N] = x16
    xT_list = [np.ascontiguousarray(x_pad[c * BPC:(c + 1) * BPC].T)
               for c in range(NCORES)]
    W1d = W1.astype(np.float16)
    W2d = np.zeros((HID, 128), np.float16)
    W2d[:, :FOUT] = W2.astype(np.float16)
    b1d = np.broadcast_to(b1, (128, HID)).astype(np.float32).copy()
    b2d = np.broadcast_to(b2, (128, FOUT)).astype(np.float32).copy()
    iota_np = np.tile(np.arange(128, dtype=np.uint8), (128, 1)).copy()
    return xT_list, W1d, W2d, b1d, b2d, iota_np


def kernel(x, edge_index, W1, b1, W2, b2, a, _want_results=False, _trace=False):
    import threading

    x = np.asarray(x, np.float32)
    edge_index = np.asarray(edge_index, np.int32)
    pre = {}

    def _run_pre():
        pre["r"] = _preprocess(edge_index)

    th = threading.Thread(target=_run_pre)
    th.start()
    xT_list, W1d, W2d, b1d, b2d, iota_np = _stage_x(
        x, np.asarray(W1, np.float32), np.asarray(b1, np.float32),
        np.asarray(W2, np.float32), np.asarray(b2, np.float32))
    th.join()
    dinv, idxs_dev, ldst_dev, Kq, C = pre["r"]

    key = (Kq, float(a))
    if key not in _cache:
        _cache[key] = _build(Kq, float(a))
    nc = _cache[key]
    in_maps = []
    for c in range(NCORES):
        lo, hi = c * BPC, (c + 1) * BPC
        in_maps.append({
            "xT": xT_list[c],
            "idxs": idxs_dev[c],
            "ldst": ldst_dev[c],
            "W1": W1d, "W2": W2d, "b1": b1d, "b2": b2d,
            "dinvb": np.ascontiguousarray(dinv[lo:hi].reshape(NBLK, 128).T),
            "iota": iota_np,
        })
    res = run_bass_kernel_spmd(nc, in_maps, core_ids=list(range(NCORES)),
                               trace=_trace)
    outs = [res.results[c]["out"] for c in range(NCORES)]
    full = np.concatenate(outs, axis=0)[:N].astype(np.float32)
    if _want_results:
        return full, res
    return full


# revision 16
# speedup vs baseline: 2.1490x; 1.2434x over previous
"""Trainium2 Bass kernel for a 2-layer GCN (GRACE encoder) on 8 NeuronCores.

Math (per layer, from the reference):
    h   = Z @ W
    deg = bincount(dst)            (self-loops included in edge list)
    dinv = deg^-1/2
    out = PReLU(segment_sum(h[src] * dinv[src] * dinv[dst], dst) + b)

We use dinv[s]*h[s] = ((dinv*Z) @ W)[s] =: P[s], so the per-edge work is a
pure row-gather of P plus a segment-sum, and all scaling is per-node:
    out = PReLU(dinv * segment_sum(P[src], dst) + b)

Sharding: dst-partitioned. Core c owns dst rows [c*12544, (c+1)*12544).
Each core computes P for its own rows, an AllGather makes the full P table
visible everywhere, and the scatter (segment-sum) is done with one-hot
selection matmuls accumulating in PSUM, 128 edges per matmul.

Tables travel as f16 (P2 zero-padded to 128 features so every gathered row
is 256 B). Edge messages are fetched with batched `dma_gather` custom DMA
instructions — int16 indices limit each gather to a 25088-row quarter of
the table, so each dst block does 4 gathers (edges are quarter-sorted).
PSUM accumulation and the PReLU epilogue stay f32.
"""

import sys

for p in ("/opt/trn_rl_repo", "/opt/trn_rl_repo/concourse"):
    if p not in sys.path:
        sys.path.insert(0, p)

import numpy as np

import jax

# Persistent XLA compilation cache: repeated kernel() calls re-trace a fresh
# closure inside run_bass_kernel_spmd; the disk cache turns the per-call
# backend compile (XLA + walrus NEFF wrap) into a hash lookup.
jax.config.update("jax_compilation_cache_dir", "/tmp/jax_comp_cache_gcn")
jax.config.update("jax_persistent_cache_min_entry_size_bytes", 0)
jax.config.update("jax_persistent_cache_min_compile_time_secs", 0)

import concourse.bass as bass
import concourse.bacc as bacc
import concourse.tile as tile
from concourse import mybir
from concourse.bass import ds
from concourse.bass_utils import run_bass_kernel_spmd
from concourse.masks import make_identity

N = 100000
E = 1600000
FIN = 128
HID = 128
FOUT = 64
NCORES = 8
BPC = 12544          # dst rows per core (padded); 8 * 12544 = 100352
NPAD = NCORES * BPC
NBLK = BPC // 128    # 98 dst blocks of 128 per core
PCH = 128            # edges per matmul chunk
NQ = 4               # source quarters (int16 gather index limit)
QS = 32768           # rows per quarter (pow2 -> shift/mask indexing)
QROWS = (32768, 32768, 32768, NPAD - 3 * 32768)   # last quarter is short

F16 = mybir.dt.float16
F32 = mybir.dt.float32
U8 = mybir.dt.uint8
I16 = mybir.dt.int16

_cache = {}


def _preprocess(edge_index):
    """Sort edges by (dst block, src); group per (dst block, src quarter)
    with per-quarter chunk counts Kq (SPMD + loop-friendly). Vectorized."""
    loops = np.arange(N, dtype=np.int32)
    src = np.concatenate([edge_index[0], loops])
    dst = np.concatenate([edge_index[1], loops])
    deg = np.bincount(dst, minlength=N).astype(np.float32)
    dinv = np.zeros(NPAD, np.float32)
    dinv[:N] = np.where(deg > 0, 1.0 / np.sqrt(deg), 0.0)

    blk = dst >> 7                              # global 128-row dst block id
    nblk_glob = NPAD // 128                     # 784
    # group key (dst block, src quarter) fits int16 -> 8x faster radix argsort;
    # within-group edge order is irrelevant (the scatter-sum is commutative)
    g4u = (blk << 2) | (src >> 15)
    order = np.argsort(g4u.astype(np.int16), kind="stable")
    g4 = g4u[order]
    src_s = src[order]
    blk_s = g4 >> 2
    ld_s = (dst & 127).astype(np.uint8)[order]
    qr_s = g4 & 3                               # src quarter (QS = 2^15)
    loc_s = (src_s & (QS - 1)).astype(np.int16)

    counts4 = np.bincount(g4, minlength=nblk_glob * NQ)
    cmax = counts4.reshape(nblk_glob, NQ).max(axis=0)
    Kq = tuple(max(1, int(np.ceil(c / PCH))) for c in cmax)
    offq = np.zeros(NQ, np.int32)
    offq[1:] = np.cumsum(Kq)[:-1]
    CB = int(sum(Kq))                           # chunks per dst block
    C = NBLK * CB                               # chunks per core per layer

    bstart = np.zeros(nblk_glob * NQ + 1, np.int32)
    bstart[1:] = np.cumsum(counts4)
    ne = len(src_s)
    rank = np.arange(ne, dtype=np.int32) - bstart[g4]
    core = blk_s // NBLK
    j = blk_s - core * NBLK
    pos = core * (C * PCH) + (j * CB + offq[qr_s]) * PCH + rank

    sa = np.zeros(NCORES * C * PCH, np.int16)
    la = np.full(NCORES * C * PCH, 255, np.uint8)
    sa[pos] = loc_s
    la[pos] = ld_s

    # idxs: per gather call (j,qr) of n=Kq[qr]*128 edges, elem i -> [i%16, i//16]
    idxs_dev = np.empty((NCORES, 16, C * 8), np.int16)
    A = sa.reshape(NCORES, NBLK, CB * PCH)
    out4 = idxs_dev.reshape(NCORES, 16, NBLK, CB * 8)
    for qr in range(NQ):
        o, k = int(offq[qr]), Kq[qr]
        seg = (A[:, :, o * PCH:(o + k) * PCH]
               .reshape(NCORES, NBLK, k * 8, 16).transpose(0, 3, 1, 2))
        out4[:, :, :, o * 8:(o + k) * 8] = seg
    # ldst: [core, C, 128] -> [core, 128, C] (partition = slot in chunk)
    ldst_dev = np.ascontiguousarray(
        la.reshape(NCORES, C, PCH).swapaxes(1, 2))

    return dinv, idxs_dev, ldst_dev, Kq, C


def _build(Kq, a_val):
    """Build the SPMD Bass program (identical on all cores)."""
    CB = int(sum(Kq))
    C = NBLK * CB
    offq = [0] * NQ
    for qr in range(1, NQ):
        offq[qr] = offq[qr - 1] + Kq[qr - 1]
    nc = bacc.Bacc("TRN2", target_bir_lowering=False, debug=False,
                   num_devices=NCORES)

    xT = nc.dram_tensor("xT", [128, BPC], F16, kind="ExternalInput")
    idxs = nc.dram_tensor("idxs", [16, C * 8], I16, kind="ExternalInput")
    ldst = nc.dram_tensor("ldst", [128, C], U8, kind="ExternalInput")
    W1 = nc.dram_tensor("W1", [FIN, HID], F16, kind="ExternalInput")
    W2 = nc.dram_tensor("W2", [HID, 128], F16, kind="ExternalInput")  # 0-padded
    b1 = nc.dram_tensor("b1", [128, HID], F32, kind="ExternalInput")
    b2 = nc.dram_tensor("b2", [128, FOUT], F32, kind="ExternalInput")
    dinvb = nc.dram_tensor("dinvb", [128, NBLK], F32, kind="ExternalInput")
    iota = nc.dram_tensor("iota", [128, 128], U8, kind="ExternalInput")
    out = nc.dram_tensor("out", [BPC, FOUT], F16, kind="ExternalOutput")

    P1_my = nc.dram_tensor("P1_my", [BPC, HID], F16, kind="Internal")
    P1_full = nc.dram_tensor("P1_full", [NPAD, HID], F16, kind="Internal",
                             addr_space="Shared")
    P2_my = nc.dram_tensor("P2_my", [BPC, 128], F16, kind="Internal")
    P2_full = nc.dram_tensor("P2_full", [NPAD, 128], F16, kind="Internal",
                             addr_space="Shared")

    with tile.TileContext(nc) as tc:
        with (
            tc.tile_pool(name="persist", bufs=1) as pp,
            tc.tile_pool(name="work", bufs=4) as wp,
            tc.tile_pool(name="gath", bufs=4) as gp,
            tc.tile_pool(name="psA", bufs=2, space="PSUM") as psA,
            tc.tile_pool(name="psB", bufs=2, space="PSUM") as psB,
        ):
            # ---- persistent SBUF state ----
            xT_sb = pp.tile([128, BPC], F16)
            nc.sync.dma_start(out=xT_sb[:], in_=xT[:])
            idx_sb = pp.tile([128, C * 8], I16)
            nc.sync.dma_start(out=idx_sb[0:16, :], in_=idxs[:])
            # replicate the 16-partition wrap across all 8 gpsimd core groups
            nc.sync.dma_start(out=idx_sb[16:32, :], in_=idx_sb[0:16, :])
            nc.sync.dma_start(out=idx_sb[32:64, :], in_=idx_sb[0:32, :])
            nc.sync.dma_start(out=idx_sb[64:128, :], in_=idx_sb[0:64, :])
            ldst_sb = pp.tile([128, C], U8)
            nc.sync.dma_start(out=ldst_sb[:], in_=ldst[:])
            W1_sb = pp.tile([FIN, HID], F16)
            nc.sync.dma_start(out=W1_sb[:], in_=W1[:])
            W2_sb = pp.tile([HID, 128], F16)
            nc.sync.dma_start(out=W2_sb[:], in_=W2[:])
            b1_sb = pp.tile([128, HID], F32)
            nc.sync.dma_start(out=b1_sb[:], in_=b1[:])
            b2_sb = pp.tile([128, FOUT], F32)
            nc.sync.dma_start(out=b2_sb[:], in_=b2[:])
            dinv_sb = pp.tile([128, NBLK], F32)
            nc.sync.dma_start(out=dinv_sb[:], in_=dinvb[:])
            iota_sb = pp.tile([128, 128], U8)
            nc.sync.dma_start(out=iota_sb[:], in_=iota[:])
            ident_sb = pp.tile([128, 128], F16)
            make_identity(nc, ident_sb[:])
            h1T_sb = pp.tile([128, BPC], F16)   # transposed layer-1 output

            # ---- phase A: P1 = dinv * (x @ W1), own shard ----
            # (python-unrolled: matmul lhsT cannot take a register offset)
            for j in range(NBLK):
                ps = psA.tile([128, HID], F32, tag="pcomp")
                nc.tensor.matmul(out=ps[:], lhsT=xT_sb[:, j * 128:(j + 1) * 128],
                                 rhs=W1_sb[:], start=True, stop=True)
                p1t = wp.tile([128, HID], F16, tag="ptile")
                nc.vector.tensor_scalar_mul(p1t[:], ps[:], dinv_sb[:, j:j + 1])
                nc.sync.dma_start(out=P1_my[j * 128:(j + 1) * 128, :], in_=p1t[:])

            nc.gpsimd.collective_compute(
                "AllGather", mybir.AluOpType.bypass,
                replica_groups=[list(range(NCORES))],
                ins=[P1_my[:]], outs=[P1_full[:]],
            )

            def gather_block(i, table, msgtag):
                msg = gp.tile([128, CB * 128], F16, tag=msgtag)
                for qr in range(NQ):
                    k = Kq[qr]
                    o = offq[qr]
                    nc.gpsimd.dma_gather(
                        out_ap=msg[:, o * 128:(o + k) * 128]
                            .rearrange("p (a b) -> p a b", a=k),
                        in_ap=table[qr * QS:qr * QS + QROWS[qr], :],
                        idxs_ap=idx_sb[:, ds(i * (CB * 8) + o * 8, k * 8)],
                        num_idxs=k * 128,
                        num_idxs_reg=k * 128,
                        elem_size=128,
                    )
                return msg

            def sel_block(i):
                selg = wp.tile([128, CB * 128], F16, tag="selg")
                nc.vector.tensor_tensor(
                    out=selg[:].rearrange("p (a b) -> p a b", a=CB),
                    in0=ldst_sb[:, ds(i * CB, CB), None]
                        .to_broadcast([128, CB, 128]),
                    in1=iota_sb[:, None, :].to_broadcast([128, CB, 128]),
                    op=mybir.AluOpType.is_equal)
                return selg

            # ---- phase B: layer-1 gather + scatter matmuls ----
            with tc.For_i(0, NBLK) as i:
                selg = sel_block(i)
                msg = gather_block(i, P1_full, "msg1")
                agg = psA.tile([128, HID], F32, tag="agg")
                for q in range(CB):
                    nc.tensor.matmul(out=agg[:], lhsT=selg[:, q * 128:(q + 1) * 128],
                                     rhs=msg[:, q * 128:q * 128 + HID],
                                     start=(q == 0), stop=(q == CB - 1))
                # finalize: h1 = PReLU(dinv*agg + b1)
                z = wp.tile([128, HID], F32, tag="z1")
                nc.vector.tensor_scalar_mul(z[:], agg[:], dinv_sb[:, ds(i, 1)])
                nc.vector.tensor_tensor(out=z[:], in0=z[:], in1=b1_sb[:],
                                        op=mybir.AluOpType.add)
                za = wp.tile([128, HID], F32, tag="za1")
                nc.vector.tensor_scalar_mul(za[:], z[:], float(a_val))
                h1 = wp.tile([128, HID], F16, tag="h1")
                nc.vector.tensor_tensor(out=h1[:], in0=z[:], in1=za[:],
                                        op=mybir.AluOpType.max)
                # transpose for the layer-2 P matmul
                pt = psB.tile([128, 128], F16, tag="tpose")
                nc.tensor.transpose(out=pt[:], in_=h1[:], identity=ident_sb[:])
                nc.vector.tensor_copy(h1T_sb[:, ds(i * 128, 128)], pt[:])

            # ---- phase C: P2 = dinv * (h1 @ W2pad), own shard ----
            # (python-unrolled: matmul lhsT cannot take a register offset)
            for j in range(NBLK):
                ps = psA.tile([128, 128], F32, tag="pcomp")
                nc.tensor.matmul(out=ps[:], lhsT=h1T_sb[:, j * 128:(j + 1) * 128],
                                 rhs=W2_sb[:], start=True, stop=True)
                p2t = wp.tile([128, 128], F16, tag="ptile")
                nc.vector.tensor_scalar_mul(p2t[:], ps[:], dinv_sb[:, j:j + 1])
                nc.sync.dma_start(out=P2_my[j * 128:(j + 1) * 128, :], in_=p2t[:])

            nc.gpsimd.collective_compute(
                "AllGather", mybir.AluOpType.bypass,
                replica_groups=[list(range(NCORES))],
                ins=[P2_my[:]], outs=[P2_full[:]],
            )

            # ---- phase D: layer-2 gather + scatter + finalize ----
            with tc.For_i(0, NBLK) as i:
                selg = sel_block(i)
                msg = gather_block(i, P2_full, "msg2")
                agg = psA.tile([128, FOUT], F32, tag="agg")
                for q in range(CB):
                    nc.tensor.matmul(out=agg[:], lhsT=selg[:, q * 128:(q + 1) * 128],
                                     rhs=msg[:, q * 128:q * 128 + FOUT],
                                     start=(q == 0), stop=(q == CB - 1))
                z = wp.tile([128, FOUT], F32, tag="z2")
                nc.vector.tensor_scalar_mul(z[:], agg[:], dinv_sb[:, ds(i, 1)])
                nc.vector.tensor_tensor(out=z[:], in0=z[:], in1=b2_sb[:],
                                        op=mybir.AluOpType.add)
                za = wp.tile([128, FOUT], F32, tag="za2")
                nc.vector.tensor_scalar_mul(za[:], z[:], float(a_val))
                yo = wp.tile([128, FOUT], F16, tag="yo")
                nc.vector.tensor_tensor(out=yo[:], in0=z[:], in1=za[:],
                                        op=mybir.AluOpType.max)
                nc.sync.dma_start(out=out[ds(i * 128, 128), :], in_=yo[:])

    nc.compile()
    return nc


def _stage_x(x, W1, b1, W2, b2):
    """Edge-independent staging (overlaps with _preprocess in a thread)."""
    x16 = x.astype(np.float16)
    x_pad = np.zeros((NPAD, FIN), np.float16)
    x_pad[:N] = x16
    xT_list = [np.ascontiguousarray(x_pad[c * BPC:(c + 1) * BPC].T)
               for c in range(NCORES)]
    W1d = W1.astype(np.float16)
    W2d = np.zeros((HID, 128), np.float16)
    W2d[:, :FOUT] = W2.astype(np.float16)
    b1d = np.broadcast_to(b1, (128, HID)).astype(np.float32).copy()
    b2d = np.broadcast_to(b2, (128, FOUT)).astype(np.float32).copy()
    iota_np = np.tile(np.arange(128, dtype=np.uint8), (128, 1)).copy()
    return xT_list, W1d, W2d, b1d, b2d, iota_np


def kernel(x, edge_index, W1, b1, W2, b2, a, _want_results=False, _trace=False):
    x = np.asarray(x, np.float32)
    edge_index = np.asarray(edge_index, np.int32)
    dinv, idxs_dev, ldst_dev, Kq, C = _preprocess(edge_index)
    xT_list, W1d, W2d, b1d, b2d, iota_np = _stage_x(
        x, np.asarray(W1, np.float32), np.asarray(b1, np.float32),
        np.asarray(W2, np.float32), np.asarray(b2, np.float32))

    key = (Kq, float(a))
    if key not in _cache:
        _cache[key] = _build(Kq, float(a))
    nc = _cache[key]
    in_maps = []
    for c in range(NCORES):
        lo, hi = c * BPC, (c + 1) * BPC
        in_maps.append({
            "xT": xT_list[c],
            "idxs": idxs_dev[c],
            "ldst": ldst_dev[c],
            "W1": W1d, "W2": W2d, "b1": b1d, "b2": b2d,
            "dinvb": np.ascontiguousarray(dinv[lo:hi].reshape(NBLK, 128).T),
            "iota": iota_np,
        })
    res = run_bass_kernel_spmd(nc, in_maps, core_ids=list(range(NCORES)),
                               trace=_trace)
    outs = [res.results[c]["out"] for c in range(NCORES)]
    full = np.concatenate(outs, axis=0)[:N].astype(np.float32)
    if _want_results:
        return full, res
    return full
